# revision 69
# baseline (speedup 1.0000x reference)
"""Trainium2 Bass kernel for nn_AttentionBlock (B=8, C=512, H=W=32, heads=8, groups=32).

Sharding: data-parallel over batch B across the 8 NeuronCores (1 batch element
per core, no collectives). Each core computes, for its X slice [512, 1024]:

    GroupNorm -> qkv 1x1 conv -> 8-head attention (S=1024, hd=64) -> proj -> +residual

Key layout choices:
  - qkv / proj 1x1-conv matmuls run fp8(e4m3) with DoubleRow perf mode
    (256-deep contraction per pass); attention stays bf16. fp8 operands use
    the DoubleRow layout [128, blk, sub, ...], contraction index
    c = blk*256 + sub*128 + partition.
  - X, Xn, Q, K channel-major [C, S]; V produced pre-transposed as [S, C_v]
    by swapping matmul operands, so attention needs no explicit transposes.
  - scores^T[k, q] per head via K=64 matmuls, two heads packed in the PE
    array with row-tiling (heads 2p/2p+1 in partitions 0-63/64-127).
  - softmax exp split across engines: 6 of 8 key-chunks per unit on the
    scalar engine (exact exp from PSUM, 1/sqrt(hd) scale fused, bf16 out),
    the last 2 chunks on the DVE via the Schraudolph bit-trick
    (round(score*scale*log2e*128 + (16256-7.5)) as int16, bitcast to bf16;
    ~1.8% rms exp error, absorbed by the softmax self-normalization).
  - attn@V runs as matmuls against [V | 1] blocks (M=65): row 64 of the
    PSUM output is the softmax denominator for free.
  - denominators are reciprocal'd with the fast DVE op after a DMA
    re-layout over 128 partitions, then DMA-broadcast and multiplied into
    the attention output on the GpSimd engine (softmax normalize, fp8 out
    for the DoubleRow proj).
  - Q/K matmuls of head-pair p+1 ride along inside pair p's attention
    stream (sharing the scores psum pool) to keep the PE dense.
  - dummy warmup matmuls during the input-DMA dead time hold the PE HAM
    clock gate open (2.4 GHz) for the phase-1 matmuls.
  - proj bias + residual fused into one scalar_tensor_tensor eviction.
"""
import numpy as np
import ml_dtypes
from contextlib import ExitStack

import concourse.bacc as bacc
import concourse.bass as bass
import concourse.tile as tile
from concourse import mybir
from concourse.bass_utils import run_bass_kernel_spmd

F32 = mybir.dt.float32
F32R = mybir.dt.float32r
BF16 = mybir.dt.bfloat16
F8 = mybir.dt.float8e4
I16 = mybir.dt.int16
AF = mybir.ActivationFunctionType
DR = mybir.MatmulPerfMode.DoubleRow

B, C, H, W = 8, 512, 32, 32
S = H * W            # 1024
NH = 8               # heads
HD = C // NH         # 64
NG = 32              # groups
GS = C // NG         # 16 channels per group
EPS = 1e-5
NCC = C // 128       # 4 channel chunks
NC2 = C // 256       # 2 channel double-chunks (DoubleRow)
NSC = S // 128       # 8 sequence chunks of 128
NS2 = S // 256       # 4 sequence double-chunks
NQ = S // 512        # 2 q-chunks of 512
SCALE = HD ** -0.5   # 0.125
# exp work split: the FIRST NDV kc chunks of each unit go to the DVE via the
# Schraudolph bit-trick (int16 bits viewed as bf16) so the scalar engine's
# exact-exp load drops below the PE/DVE time; attn@V runs bf16 throughout.
NDV = 2              # DVE-handled kc chunks (at the tail of each unit)
# Schraudolph constants: bits = rne(score*SCALE*log2e*128 + (16256 - 7.5))
SCH1 = float(SCALE * np.log2(np.e) * 128.0)
SCH2 = 16256.0 - 7.5


def build_nc():
    nc = bacc.Bacc("TRN2", target_bir_lowering=False, debug=False)

    # ---- DRAM parameters (per-core). Declaration order = binding order.
    x_d = nc.declare_dram_parameter("x", [C, S], F32, isOutput=False)
    qkvw_d = nc.declare_dram_parameter("qkv_w8", [128, NC2, 2, 3 * C], F8,
                                       isOutput=False)
    projw_d = nc.declare_dram_parameter("proj_w8", [128, NC2, 2, C], F8,
                                        isOutput=False)
    gsum_d = nc.declare_dram_parameter("gsum", [C, NG], F32R, isOutput=False)
    gexp_d = nc.declare_dram_parameter("gexpT", [NG, C], F32R, isOutput=False)
    smc_d = nc.declare_dram_parameter("small_consts", [128, 3 * NCC + 12],
                                      F32, isOutput=False)
    y_d = nc.declare_dram_parameter("y", [C, S], F32, isOutput=True)

    # DRAM scratch for the softmax-denominator reciprocal broadcast.
    # layout [pair][qn][head-in-pair][q512]
    recip_d = nc.dram_tensor("recip_scratch", [NH // 2, NQ, 2, 512], F32)

    with tile.TileContext(nc) as tc, ExitStack() as ctx:
        const = ctx.enter_context(tc.tile_pool(name="const", bufs=1))
        xp = ctx.enter_context(tc.tile_pool(name="xp", bufs=1))
        qp = ctx.enter_context(tc.tile_pool(name="qp", bufs=1))
        kp = ctx.enter_context(tc.tile_pool(name="kp", bufs=1))
        vp = ctx.enter_context(tc.tile_pool(name="vp", bufs=1))
        anp = ctx.enter_context(tc.tile_pool(name="anp", bufs=1))
        outp = ctx.enter_context(tc.tile_pool(name="outp", bufs=2))
        pwp = ctx.enter_context(tc.tile_pool(name="pwp", bufs=1))
        xnp = ctx.enter_context(tc.tile_pool(name="xnp", bufs=1))
        wqp = ctx.enter_context(tc.tile_pool(name="wqp", bufs=1))
        gnp = ctx.enter_context(tc.tile_pool(name="gnp", bufs=1))

        # ---------- load X first (GN stats gate everything) ----------
        x_sb = [xp.tile([128, S], F32, tag=f"x{cc}", name=f"x{cc}")
                for cc in range(NCC)]
        for cc in range(NCC):
            deng = nc.sync if cc < 2 else nc.gpsimd
            deng.dma_start(x_sb[cc][:], x_d[128 * cc:128 * (cc + 1), :])
        gsum_sb = gnp.tile([C // NCC, NG * NCC], F32R)
        nc.sync.dma_start(
            gsum_sb[:].rearrange("p (cc g) -> p cc g", cc=NCC),
            gsum_d[:].rearrange("(cc p) g -> p cc g", cc=NCC))
        # ---------- constants ----------
        smc_sb = const.tile([128, 3 * NCC + 12], F32)
        nc.sync.dma_start(smc_sb[:], smc_d[:])
        w4_sb = smc_sb[:, 0:NCC]
        b4_sb = smc_sb[:, NCC:2 * NCC]
        pb_sb = smc_sb[:, 2 * NCC:3 * NCC]
        # dedicated tile for the qkv bias: scalar.activation bias APs
        # mis-offset into packed-tile slices, so ACT reads need a real tile
        qb_sb = const.tile([128, 12], F32, name="qb_sb")
        nc.vector.tensor_copy(qb_sb[:], smc_sb[:, 3 * NCC:3 * NCC + 12])
        gexp_sb = const.tile([NG, C], F32R)
        nc.sync.dma_start(gexp_sb[:], gexp_d[:])


        qkvw_sb = wqp.tile([128, NC2, 2, 3 * C], F8)
        nc.sync.dma_start(qkvw_sb[:], qkvw_d[:])
        pw_sb = pwp.tile([128, NC2, 2, C], F8)
        nc.sync.dma_start(pw_sb[:], projw_d[:])

        q_sb = [qp.tile([128, S], BF16, tag=f"q{p}", name=f"q{p}")
                for p in range(NH // 2)]
        k_sb = [kp.tile([128, S], BF16, tag=f"k{p}", name=f"k{p}")
                for p in range(NH // 2)]
        # [64 v-channels | 1.0] per head block: the ones column turns the
        # attn@V matmul (M=65) into attn@V plus the softmax denominator.
        vT_sb = [vp.tile([128, 65 * NH], BF16, tag=f"v{sc}", name=f"v{sc}")
                 for sc in range(NSC)]
        # softmax-normalized attention out, DoubleRow layout for proj
        an_sb = anp.tile([128, NC2, 2, S], F8)
        # GN output in DoubleRow layout for qkv matmuls
        xn_sb = xnp.tile([128, NC2, 2, S], F8)

        # load the ln/exp ACT table set while the input DMAs run
        warm = gnp.tile([1, 1], F32)
        nc.vector.memset(warm[:], 1.0)
        nc.scalar.activation(out=warm[:], in_=warm[:], func=AF.Ln,
                             bias=warm[:], scale=1.0)
        # PE HAM warmup: the clock gate defaults to 1.2 GHz and only opens to
        # 2.4 GHz after ~3.4us of sustained matmul activity. Burn dummy
        # matmuls during the DMA/stats dead time so phase-1 matmuls run warm.
        warm_w = const.tile([128, 128], BF16)
        nc.vector.memset(warm_w[:], 0.0)

        # ================= phase 1: GN + V^T + Q/K of pair 0 ================
        with ExitStack() as ph1:
            xsqp = ph1.enter_context(tc.tile_pool(name="xsqp", bufs=2))
            warm_ps = ph1.enter_context(
                tc.tile_pool(name="warm_ps", bufs=1, space="PSUM"))
            ps_w = warm_ps.tile([128, 128], F32, tag="warm")
            for _ in range(48):
                nc.tensor.matmul(ps_w[:], warm_w[:], warm_w[:],
                                 start=True, stop=True)
            gn_es = ExitStack()
            gn_ps = gn_es.enter_context(
                tc.tile_pool(name="gn_ps", bufs=1, space="PSUM"))
            small_ps = gn_es.enter_context(
                tc.tile_pool(name="small_ps", bufs=2, space="PSUM"))

            # ---------- GroupNorm stats ----------
            # per-channel sum (DVE accumulate) and sum of squares (ACT Square
            # accumulate); a tiny f32r matmul against the group map then does
            # the cross-partition group reduction.
            s12 = gnp.tile([128, 2 * NCC], F32)
            for cc in range(NCC):
                scr = xsqp.tile([128, S], BF16)
                nc.vector.scalar_tensor_tensor(
                    out=scr[:], in0=x_sb[cc][:], scalar=1.0, in1=x_sb[cc][:],
                    op0=mybir.AluOpType.mult, op1=mybir.AluOpType.bypass,
                    accum_out=s12[:, 2 * cc:2 * cc + 1])
                scr2 = xsqp.tile([128, S], BF16)
                nc.scalar.activation(
                    out=scr2[:], in_=x_sb[cc][:], func=AF.Square,
                    accum_out=s12[:, 2 * cc + 1:2 * cc + 2])
            s12r = gnp.tile([128, 2 * NCC], F32R)
            nc.vector.tensor_copy(s12r[:], s12[:])
            ps_g = gn_ps.tile([NG, 2], F32, tag="ps_g")
            for cc in range(NCC):
                nc.tensor.matmul(
                    ps_g[:], gsum_sb[:, NG * cc:NG * (cc + 1)],
                    s12r[:, 2 * cc:2 * cc + 2],
                    start=(cc == 0), stop=(cc == NCC - 1))
            inv_n = 1.0 / (GS * S)
            mean_g = gnp.tile([NG, 1], F32)
            nc.vector.tensor_scalar(out=mean_g[:], in0=ps_g[:, 0:1],
                                    scalar1=inv_n,
                                    scalar2=None, op0=mybir.AluOpType.mult)
            ex2 = gnp.tile([NG, 1], F32)
            nc.vector.tensor_scalar(out=ex2[:], in0=ps_g[:, 1:2],
                                    scalar1=inv_n,
                                    scalar2=None, op0=mybir.AluOpType.mult)
            var_g = gnp.tile([NG, 1], F32)
            # var = E[x^2] - mean^2
            nc.vector.scalar_tensor_tensor(
                out=var_g[:], in0=mean_g[:], scalar=-1.0, in1=mean_g[:],
                op0=mybir.AluOpType.mult, op1=mybir.AluOpType.mult)
            nc.vector.tensor_tensor(out=var_g[:], in0=ex2[:], in1=var_g[:],
                                    op=mybir.AluOpType.add)
            # rstd = exp(-0.5 * ln(var + eps)); ln+exp share one ACT table set
            eps_sb = gnp.tile([NG, 1], F32)
            nc.vector.memset(eps_sb[:], EPS)
            lnv = gnp.tile([NG, 1], F32)
            nc.scalar.activation(out=lnv[:], in_=var_g[:], func=AF.Ln,
                                 bias=eps_sb[:], scale=1.0)
            # stats_r[:, 0] = rstd, stats_r[:, 1] = mean  (N=2 matmul rhs)
            stats_r = gnp.tile([NG, 2], F32R)
            nc.scalar.activation(out=stats_r[:, 0:1], in_=lnv[:], func=AF.Exp,
                                 bias=0.0, scale=-0.5)
            nc.vector.tensor_copy(stats_r[:, 1:2], mean_g[:])

            # per-channel rstd/mean via tiny matmuls against the group map
            rstd_c = gnp.tile([128, NCC], F32)
            mean_c = gnp.tile([128, NCC], F32)
            for cc in range(NCC):
                ps_a = small_ps.tile([128, 2], F32, tag="alpha")
                nc.tensor.matmul(ps_a[:],
                                 gexp_sb[:, 128 * cc:128 * (cc + 1)],
                                 stats_r[:], start=True, stop=True)
                nc.vector.tensor_copy(rstd_c[:, cc:cc + 1], ps_a[:, 0:1])
                nc.vector.tensor_copy(mean_c[:, cc:cc + 1], ps_a[:, 1:2])
            # second HAM-warmup burst: bridges the PE-idle window between the
            # tiny stats matmuls and the first V^T matmuls.
            for _ in range(36):
                nc.tensor.matmul(ps_w[:], warm_w[:], warm_w[:],
                                 start=True, stop=True)
            alpha = gnp.tile([128, NCC], F32)
            nc.vector.tensor_tensor(out=alpha[:], in0=rstd_c[:], in1=w4_sb,
                                    op=mybir.AluOpType.mult)
            beta = gnp.tile([128, NCC], F32)
            nc.vector.tensor_tensor(out=beta[:], in0=alpha[:], in1=mean_c[:],
                                    op=mybir.AluOpType.mult)
            nc.vector.tensor_tensor(out=beta[:], in0=b4_sb, in1=beta[:],
                                    op=mybir.AluOpType.subtract)

            # ---------- GN apply (fp8 out, DoubleRow layout) ----------
            for cc in range(NCC):
                nc.vector.tensor_scalar(
                    out=xn_sb[:, cc // 2, cc % 2, :], in0=x_sb[cc][:],
                    scalar1=alpha[:, cc:cc + 1], scalar2=beta[:, cc:cc + 1],
                    op0=mybir.AluOpType.mult, op1=mybir.AluOpType.add)

            gn_es.close()
            qkv_ps = ph1.enter_context(
                tc.tile_pool(name="qkv_ps", bufs=2, space="PSUM"))

            # ---------- V^T (pre-transposed): out[s, vch], DoubleRow ----------
            for sc in range(NSC):
                ps_v = qkv_ps.tile([128, 512], F32, tag="psv")
                for c2 in range(NC2):
                    nc.tensor.matmul(
                        ps_v[:],
                        xn_sb[:, c2, :, 128 * sc:128 * (sc + 1)],
                        qkvw_sb[:, c2, :, 1024:1536],
                        start=(c2 == 0), stop=(c2 == NC2 - 1),
                        perf_mode=DR)
                vT_v = vT_sb[sc][:].rearrange("p (h u) -> p h u", u=65)
                nc.scalar.activation(
                    out=vT_v[:, :, 0:64],
                    in_=ps_v[:].rearrange("p (h u) -> p h u", u=64),
                    func=AF.Identity, bias=0.0, scale=1.0)
                nc.vector.memset(vT_v[:, :, 64:65], 1.0)

            # ---------- Q and K of pair 0, channel-major, DoubleRow ----------
            for oc in range(1):
                ps_q = qkv_ps.tile([128, S], F32, tag="psqk")
                for c2 in range(NC2):
                    for qn in range(NQ):
                        nc.tensor.matmul(
                            ps_q[:, 512 * qn:512 * (qn + 1)],
                            qkvw_sb[:, c2, :, 128 * oc:128 * (oc + 1)],
                            xn_sb[:, c2, :, 512 * qn:512 * (qn + 1)],
                            start=(c2 == 0), stop=(c2 == NC2 - 1),
                            perf_mode=DR)
                nc.scalar.activation(out=q_sb[oc][:], in_=ps_q[:],
                                     func=AF.Identity,
                                     bias=qb_sb[:, oc:oc + 1], scale=1.0)
                ps_k = qkv_ps.tile([128, S], F32, tag="psqk")
                for c2 in range(NC2):
                    for qn in range(NQ):
                        nc.tensor.matmul(
                            ps_k[:, 512 * qn:512 * (qn + 1)],
                            qkvw_sb[:, c2, :, 512 + 128 * oc:512 + 128 * (oc + 1)],
                            xn_sb[:, c2, :, 512 * qn:512 * (qn + 1)],
                            start=(c2 == 0), stop=(c2 == NC2 - 1),
                            perf_mode=DR)
                nc.scalar.activation(out=k_sb[oc][:], in_=ps_k[:],
                                     func=AF.Identity,
                                     bias=qb_sb[:, 4 + oc:5 + oc], scale=1.0)

        # ================= phase 2: attention ================
        # Per head pair p: scores^T / exp / attn@[V|1] pipelined per
        # (qn, kc). Q/K DoubleRow matmuls of pair p+1 ride along inside
        # the kc2 loop (one per step, sharing the scores psum pool slots)
        # so the PE stays dense.
        with ExitStack() as ph_att:
            expp = ph_att.enter_context(tc.tile_pool(name="expp", bufs=3))
            rawp = ph_att.enter_context(tc.tile_pool(name="rawp", bufs=2))
            rbp = ph_att.enter_context(tc.tile_pool(name="rbp", bufs=2))
            recp = ph_att.enter_context(tc.tile_pool(name="recp", bufs=2))
            sc_ps = ph_att.enter_context(
                tc.tile_pool(name="sc_ps", bufs=3, space="PSUM"))
            av_ps = ph_att.enter_context(
                tc.tile_pool(name="av_ps", bufs=1, space="PSUM"))

            for p in range(NH // 2):
                hA, hB = 2 * p, 2 * p + 1
                for qn in range(NQ):
                    # next pair's Q (during qn0) or K (during qn1) rides along
                    nxt = p + 1
                    if nxt < NH // 2:
                        ps_nxt = sc_ps.tile([128, S], F32, tag="sc",
                                            name=f"psnxt{p}_{qn}")
                        woff = 128 * nxt if qn == 0 else 512 + 128 * nxt
                    # exp outputs: ACT chunks -> exact exp, bf16 (fp8 out
                    # would cost +160ns per activation); DVE chunks -> int16
                    # Schraudolph bits (bitcast bf16)
                    exp_t = expp.tile([128, (NSC - NDV) * S], BF16, tag="exp")
                    sch_t = (expp.tile([128, NDV * S], I16, tag="sch",
                                       name="sch_t") if NDV else None)
                    ps_av = av_ps.tile([65, S], F32, tag="av")
                    ps_avA = ps_av[:, 0:512]
                    ps_avB = ps_av[:, 512:1024]
                    for kc in range(NSC):
                        kc2, sub = kc // 2, kc % 2
                        # scores^T chunk for both heads (row-tiled pair)
                        ps_s = sc_ps.tile([128, S], F32, tag="sc")
                        nc.tensor.matmul(
                            ps_s[:, 0:512],
                            k_sb[p][0:64, 128 * kc:128 * (kc + 1)],
                            q_sb[p][0:64, 512 * qn:512 * (qn + 1)],
                            start=True, stop=True, tile_position=(0, 0))
                        nc.tensor.matmul(
                            ps_s[:, 512:1024],
                            k_sb[p][64:128, 128 * kc:128 * (kc + 1)],
                            q_sb[p][64:128, 512 * qn:512 * (qn + 1)],
                            start=True, stop=True, tile_position=(64, 0))
                        # one Q/K DoubleRow matmul of the next pair per 2 kc
                        if nxt < NH // 2 and sub == 0:
                            c2, qq = kc2 // 2, kc2 % 2
                            nc.tensor.matmul(
                                ps_nxt[:, 512 * qq:512 * (qq + 1)],
                                qkvw_sb[:, c2, :, woff:woff + 128],
                                xn_sb[:, c2, :, 512 * qq:512 * (qq + 1)],
                                start=(c2 == 0), stop=(c2 == NC2 - 1),
                                perf_mode=DR)
                        if kc < NSC - NDV:
                            # exp of both heads chunk in one ACT pass
                            # (1/sqrt(hd) scale fused)
                            ka = kc
                            eo = exp_t[:, S * ka:S * (ka + 1)]
                            nc.scalar.activation(out=eo, in_=ps_s[:],
                                                 func=AF.Exp,
                                                 bias=0.0, scale=SCALE)
                        else:
                            # Schraudolph exp on the DVE: int16 bits, read
                            # back as bf16 by the attn@V matmuls
                            kd = kc - (NSC - NDV)
                            nc.vector.tensor_scalar(
                                out=sch_t[:, S * kd:S * (kd + 1)],
                                in0=ps_s[:],
                                scalar1=SCH1, scalar2=SCH2,
                                op0=mybir.AluOpType.mult,
                                op1=mybir.AluOpType.add)
                            eo = sch_t[:, S * kd:S * (kd + 1)].bitcast(BF16)
                        # attn @ [V | 1] per head: rows 0-63 = attn@V,
                        # row 64 = softmax denominator
                        nc.tensor.matmul(
                            ps_avA[:], vT_sb[kc][:, 65 * hA:65 * (hA + 1)],
                            eo[:, 0:512],
                            start=(kc == 0), stop=(kc == NSC - 1))
                        nc.tensor.matmul(
                            ps_avB[:], vT_sb[kc][:, 65 * hB:65 * (hB + 1)],
                            eo[:, 512:1024],
                            start=(kc == 0), stop=(kc == NSC - 1))

                    # next pair's Q/K eviction (+bias)
                    if nxt < NH // 2:
                        dst = q_sb[nxt] if qn == 0 else k_sb[nxt]
                        boff = nxt if qn == 0 else 4 + nxt
                        nc.vector.tensor_scalar(
                            out=dst[:], in0=ps_nxt[:],
                            scalar1=qb_sb[:, boff:boff + 1], scalar2=None,
                            op0=mybir.AluOpType.add)

                    # evict attn@V + denominators raw (frees the psum banks)
                    raw = rawp.tile([65, S], F32, tag="raw")
                    nc.vector.tensor_copy(raw[:], ps_av[:])
                    if p < NH // 2 - 1:
                        # steady state: spread [1,1024] over 128 partitions,
                        # fast DVE reciprocal, DMA roundtrip broadcast;
                        # normalize on GpSimd (latency hidden by later units)
                        d128 = recp.tile([128, 8], F32, tag="d128")
                        nc.sync.dma_start(d128[:], raw[64:65, :])
                        r128 = recp.tile([128, 8], F32, tag="r128")
                        rscr = recp.tile([128, 8], F32, tag="rscr")
                        nc.vector.reciprocal_approx_accurate(
                            out=r128[:], in_=d128[:], scratch=rscr[:])
                        r128v = recip_d[p][qn].rearrange(
                            "h (x f) -> (h x) f", f=8)
                        nc.sync.dma_start(r128v, r128[:])
                        rbA = rbp.tile([64, 512], F32, tag="rbA")
                        rbB = rbp.tile([64, 512], F32, tag="rbB")
                        for rb_t, h in ((rbA, 0), (rbB, 1)):
                            rsrc = recip_d[p][qn][h]  # [512]
                            rsrc_b = bass.AP(tensor=rsrc.tensor,
                                             offset=rsrc.offset,
                                             ap=[[0, 64], list(rsrc.ap[0])])
                            nc.sync.dma_start(rb_t[:], rsrc_b)
                        nc.gpsimd.tensor_tensor(
                            out=an_sb[0:64, p // 2, p % 2,
                                      512 * qn:512 * (qn + 1)],
                            in0=raw[0:64, 0:512], in1=rbA[:],
                            op=mybir.AluOpType.mult)
                        nc.gpsimd.tensor_tensor(
                            out=an_sb[64:128, p // 2, p % 2,
                                      512 * qn:512 * (qn + 1)],
                            in0=raw[0:64, 512:1024], in1=rbB[:],
                            op=mybir.AluOpType.mult)
                    else:
                        # pair 3 gates the proj tail: DMA-free path. recip =
                        # exp(-ln(den)) on the (idle) ACT engine, broadcast
                        # across 64 partitions with a K=1 matmul into the
                        # unused ride-along psum, normalize on the DVE.
                        lnden = recp.tile([1, S], F32, tag="lnden")
                        nc.scalar.activation(out=lnden[:], in_=raw[64:65, :],
                                             func=AF.Ln, bias=0.0, scale=1.0)
                        rrow = recp.tile([1, S], F32R, tag="rrow")
                        nc.scalar.activation(out=rrow[:], in_=lnden[:],
                                             func=AF.Exp, bias=0.0,
                                             scale=-1.0)
                        ps_rb = nxt_ps.tile([128, S], F32, tag="nxt",
                                            name=f"psrb{qn}")
                        rrow_r = rrow[:]
                        for half in range(2):
                            nc.tensor.matmul(
                                ps_rb[0:64, 512 * half:512 * (half + 1)],
                                ones64_r[:],
                                rrow_r[:, 512 * half:512 * (half + 1)],
                                start=True, stop=True)
                        nc.vector.tensor_tensor(
                            out=an_sb[0:64, p // 2, p % 2,
                                      512 * qn:512 * (qn + 1)],
                            in0=raw[0:64, 0:512], in1=ps_rb[0:64, 0:512],
                            op=mybir.AluOpType.mult)
                        nc.vector.tensor_tensor(
                            out=an_sb[64:128, p // 2, p % 2,
                                      512 * qn:512 * (qn + 1)],
                            in0=raw[0:64, 512:1024],
                            in1=ps_rb[0:64, 512:1024],
                            op=mybir.AluOpType.mult)

        # ================= phase 3: proj + bias + residual ================
        with ExitStack() as ph_proj:
            pj_ps = ph_proj.enter_context(
                tc.tile_pool(name="pj_ps", bufs=1, space="PSUM"))
            ps_o = [pj_ps.tile([128, S], F32, tag=f"pso{oc}", name=f"pso{oc}")
                    for oc in range(NCC)]
            for c2 in range(NC2):
                for oc in range(NCC):
                    for qn in range(NQ):
                        nc.tensor.matmul(
                            ps_o[oc][:, 512 * qn:512 * (qn + 1)],
                            pw_sb[:, c2, :, 128 * oc:128 * (oc + 1)],
                            an_sb[:, c2, :, 512 * qn:512 * (qn + 1)],
                            start=(c2 == 0), stop=(c2 == NC2 - 1),
                            perf_mode=DR)
            # evictions: bias + residual fused in one DVE op per chunk,
            # output DMA issued immediately per chunk
            for oc in range(NCC):
                out_t = outp.tile([128, S], F32, tag="out")
                nc.vector.scalar_tensor_tensor(
                    out=out_t[:], in0=ps_o[oc][:],
                    scalar=pb_sb[:, oc:oc + 1], in1=x_sb[oc][:],
                    op0=mybir.AluOpType.add, op1=mybir.AluOpType.add)
                nc.sync.dma_start(y_d[128 * oc:128 * (oc + 1), :], out_t[:])

    nc.finalize()
    return nc


_NC_CACHE = None


def _get_nc():
    global _NC_CACHE
    if _NC_CACHE is None:
        _NC_CACHE = build_nc()
    return _NC_CACHE


def make_in_maps(X, norm_w, norm_b, qkv_w, qkv_b, proj_w, proj_b):
    X = np.asarray(X, dtype=np.float32)
    norm_w = np.asarray(norm_w, dtype=np.float32)
    norm_b = np.asarray(norm_b, dtype=np.float32)
    qkv_w = np.asarray(qkv_w, dtype=np.float32)
    qkv_b = np.asarray(qkv_b, dtype=np.float32)
    proj_w = np.asarray(proj_w, dtype=np.float32)
    proj_b = np.asarray(proj_b, dtype=np.float32)

    # DoubleRow weight layout: [part, c2, sub, out] with
    # c = c2*256 + sub*128 + part
    qkv_w8 = np.ascontiguousarray(
        qkv_w.T.reshape(NC2, 2, 128, 3 * C).transpose(2, 0, 1, 3)
    ).astype(ml_dtypes.float8_e4m3)
    proj_w8 = np.ascontiguousarray(
        proj_w.T.reshape(NC2, 2, 128, C).transpose(2, 0, 1, 3)
    ).astype(ml_dtypes.float8_e4m3)
    gsum = np.zeros((C, NG), np.float32)
    gsum[np.arange(C), np.arange(C) // GS] = 1.0
    gexpT = np.ascontiguousarray(gsum.T)                      # [32, 512]
    w4 = np.ascontiguousarray(norm_w.reshape(NCC, 128).T)     # [128, 4]
    b4 = np.ascontiguousarray(norm_b.reshape(NCC, 128).T)
    qb12 = np.ascontiguousarray(qkv_b.reshape(12, 128).T)     # [128, 12]
    # attn(V + vb) = attn(V) + vb (softmax weights sum to 1), and proj is
    # linear, so the V bias folds into the proj bias exactly.
    pb_eff = proj_b + proj_w @ qkv_b[2 * C:3 * C]
    pb4 = np.ascontiguousarray(pb_eff.reshape(NCC, 128).T)

    small_consts = np.ascontiguousarray(
        np.concatenate([w4, b4, pb4, qb12], axis=1))
    shared = {
        "qkv_w8": qkv_w8, "proj_w8": proj_w8, "gsum": gsum, "gexpT": gexpT,
        "small_consts": small_consts,
    }
    in_maps = []
    for b in range(B):
        m = dict(shared)
        m["x"] = np.ascontiguousarray(X[b].reshape(C, S))
        in_maps.append(m)
    return in_maps


def kernel(X, norm_w, norm_b, qkv_w, qkv_b, proj_w, proj_b):
    nc = _get_nc()
    in_maps = make_in_maps(X, norm_w, norm_b, qkv_w, qkv_b, proj_w, proj_b)
    res = run_bass_kernel_spmd(nc, in_maps, core_ids=list(range(B)))
    out = np.stack([res.results[b]["y"].reshape(C, H, W) for b in range(B)])
    return out.astype(np.float32)


# revision 70
# speedup vs baseline: 1.2054x; 1.2054x over previous
"""Trainium2 Bass kernel for nn_AttentionBlock (B=8, C=512, H=W=32, heads=8, groups=32).

Sharding: data-parallel over batch B across the 8 NeuronCores (1 batch element
per core, no collectives). Each core computes, for its X slice [512, 1024]:

    GroupNorm -> qkv 1x1 conv -> 8-head attention (S=1024, hd=64) -> proj -> +residual

Key layout choices:
  - qkv / proj 1x1-conv matmuls run fp8(e4m3) with DoubleRow perf mode
    (256-deep contraction per pass); attention stays bf16. fp8 operands use
    the DoubleRow layout [128, blk, sub, ...], contraction index
    c = blk*256 + sub*128 + partition.
  - X, Xn, Q, K channel-major [C, S]; V produced pre-transposed as [S, C_v]
    by swapping matmul operands, so attention needs no explicit transposes.
  - scores^T[k, q] per head via K=64 matmuls, two heads packed in the PE
    array with row-tiling (heads 2p/2p+1 in partitions 0-63/64-127).
  - softmax exp split across engines: 6 of 8 key-chunks per unit on the
    scalar engine (exact exp from PSUM, 1/sqrt(hd) scale fused, bf16 out),
    the last 2 chunks on the DVE via the Schraudolph bit-trick
    (round(score*scale*log2e*128 + (16256-7.5)) as int16, bitcast to bf16;
    ~1.8% rms exp error, absorbed by the softmax self-normalization).
  - attn@V runs as matmuls against [V | 1] blocks (M=65): row 64 of the
    PSUM output is the softmax denominator for free.
  - denominators are reciprocal'd with the fast DVE op after a DMA
    re-layout over 128 partitions, then DMA-broadcast and multiplied into
    the attention output on the GpSimd engine (softmax normalize, fp8 out
    for the DoubleRow proj).
  - Q/K matmuls of head-pair p+1 ride along inside pair p's attention
    stream (sharing the scores psum pool) to keep the PE dense.
  - dummy warmup matmuls during the input-DMA dead time hold the PE HAM
    clock gate open (2.4 GHz) for the phase-1 matmuls.
  - proj bias + residual fused into one scalar_tensor_tensor eviction.
"""
import numpy as np
import ml_dtypes
from contextlib import ExitStack

import concourse.bacc as bacc
import concourse.bass as bass
import concourse.tile as tile
from concourse import mybir
from concourse.bass_utils import run_bass_kernel_spmd

F32 = mybir.dt.float32
F32R = mybir.dt.float32r
BF16 = mybir.dt.bfloat16
F8 = mybir.dt.float8e4
I16 = mybir.dt.int16
AF = mybir.ActivationFunctionType
DR = mybir.MatmulPerfMode.DoubleRow

B, C, H, W = 8, 512, 32, 32
S = H * W            # 1024
NH = 8               # heads
HD = C // NH         # 64
NG = 32              # groups
GS = C // NG         # 16 channels per group
EPS = 1e-5
NCC = C // 128       # 4 channel chunks
NC2 = C // 256       # 2 channel double-chunks (DoubleRow)
NSC = S // 128       # 8 sequence chunks of 128
NS2 = S // 256       # 4 sequence double-chunks
NQ = S // 512        # 2 q-chunks of 512
SCALE = HD ** -0.5   # 0.125
# exp work split: the FIRST NDV kc chunks of each unit go to the DVE via the
# Schraudolph bit-trick (int16 bits viewed as bf16) so the scalar engine's
# exact-exp load drops below the PE/DVE time; attn@V runs bf16 throughout.
NDV = 2              # DVE-handled kc chunks (at the tail of each unit)
# Schraudolph constants: bits = rne(score*SCALE*log2e*128 + (16256 - 7.5))
SCH1 = float(SCALE * np.log2(np.e) * 128.0)
SCH2 = 16256.0 - 7.5


def build_nc():
    nc = bacc.Bacc("TRN2", target_bir_lowering=False, debug=False)

    # ---- DRAM parameters (per-core). Declaration order = binding order.
    x_d = nc.declare_dram_parameter("x", [C, S], F32, isOutput=False)
    qkvw_d = nc.declare_dram_parameter("qkv_w8", [128, NC2, 2, 3 * C], F8,
                                       isOutput=False)
    projw_d = nc.declare_dram_parameter("proj_w8", [128, NC2, 2, C], F8,
                                        isOutput=False)
    gsum_d = nc.declare_dram_parameter("gsum", [C, NG], F32R, isOutput=False)
    gexp_d = nc.declare_dram_parameter("gexpT", [NG, C], F32R, isOutput=False)
    smc_d = nc.declare_dram_parameter("small_consts", [128, 3 * NCC + 12],
                                      F32, isOutput=False)
    vb_d = nc.declare_dram_parameter("vb_bcast", [128, C], F32, isOutput=False)
    y_d = nc.declare_dram_parameter("y", [C, S], F32, isOutput=True)

    # DRAM scratch for the softmax-denominator reciprocal broadcast.
    # layout [pair][qn][head-in-pair][q512]
    recip_d = nc.dram_tensor("recip_scratch", [NH // 2, NQ, 2, 512], F32)

    with tile.TileContext(nc) as tc, ExitStack() as ctx:
        const = ctx.enter_context(tc.tile_pool(name="const", bufs=1))
        xp = ctx.enter_context(tc.tile_pool(name="xp", bufs=1))
        qp = ctx.enter_context(tc.tile_pool(name="qp", bufs=1))
        kp = ctx.enter_context(tc.tile_pool(name="kp", bufs=1))
        vp = ctx.enter_context(tc.tile_pool(name="vp", bufs=1))
        anp = ctx.enter_context(tc.tile_pool(name="anp", bufs=1))
        outp = ctx.enter_context(tc.tile_pool(name="outp", bufs=2))
        pwp = ctx.enter_context(tc.tile_pool(name="pwp", bufs=1))
        xnp = ctx.enter_context(tc.tile_pool(name="xnp", bufs=1))
        wqp = ctx.enter_context(tc.tile_pool(name="wqp", bufs=1))
        gnp = ctx.enter_context(tc.tile_pool(name="gnp", bufs=1))

        # ---------- load X first (GN stats gate everything) ----------
        x_sb = [xp.tile([128, S], F32, tag=f"x{cc}", name=f"x{cc}")
                for cc in range(NCC)]
        for cc in range(NCC):
            deng = nc.sync if cc < 2 else nc.gpsimd
            deng.dma_start(x_sb[cc][:], x_d[128 * cc:128 * (cc + 1), :])
        gsum_sb = gnp.tile([C // NCC, NG * NCC], F32R)
        nc.sync.dma_start(
            gsum_sb[:].rearrange("p (cc g) -> p cc g", cc=NCC),
            gsum_d[:].rearrange("(cc p) g -> p cc g", cc=NCC))
        # ---------- constants ----------
        vb_sb = const.tile([128, C], F32)
        nc.sync.dma_start(vb_sb[:], vb_d[:])
        smc_sb = const.tile([128, 3 * NCC + 12], F32)
        nc.sync.dma_start(smc_sb[:], smc_d[:])
        w4_sb = smc_sb[:, 0:NCC]
        b4_sb = smc_sb[:, NCC:2 * NCC]
        pb_sb = smc_sb[:, 2 * NCC:3 * NCC]
        # dedicated tile for the qkv bias: scalar.activation bias APs
        # mis-offset into packed-tile slices, so ACT reads need a real tile
        qb_sb = const.tile([128, 12], F32, name="qb_sb")
        nc.vector.tensor_copy(qb_sb[:], smc_sb[:, 3 * NCC:3 * NCC + 12])
        gexp_sb = const.tile([NG, C], F32R)
        nc.sync.dma_start(gexp_sb[:], gexp_d[:])


        qkvw_sb = wqp.tile([128, NC2, 2, 3 * C], F8)
        nc.sync.dma_start(qkvw_sb[:], qkvw_d[:])
        pw_sb = pwp.tile([128, NC2, 2, C], F8)
        nc.sync.dma_start(pw_sb[:], projw_d[:])

        q_sb = [qp.tile([128, S], BF16, tag=f"q{p}", name=f"q{p}")
                for p in range(NH // 2)]
        k_sb = [kp.tile([128, S], BF16, tag=f"k{p}", name=f"k{p}")
                for p in range(NH // 2)]
        # [64 v-channels | 1.0] per head block: the ones column turns the
        # attn@V matmul (M=65) into attn@V plus the softmax denominator.
        vT_sb = [vp.tile([128, 65 * NH], BF16, tag=f"v{sc}", name=f"v{sc}")
                 for sc in range(NSC)]
        # softmax-normalized attention out, DoubleRow layout for proj
        an_sb = anp.tile([128, NC2, 2, S], F8)
        # GN output in DoubleRow layout for qkv matmuls
        xn_sb = xnp.tile([128, NC2, 2, S], F8)

        # load the ln/exp ACT table set while the input DMAs run
        warm = gnp.tile([1, 1], F32)
        nc.vector.memset(warm[:], 1.0)
        nc.scalar.activation(out=warm[:], in_=warm[:], func=AF.Ln,
                             bias=warm[:], scale=1.0)
        # PE HAM warmup: the clock gate defaults to 1.2 GHz and only opens to
        # 2.4 GHz after ~3.4us of sustained matmul activity. Burn dummy
        # matmuls during the DMA/stats dead time so phase-1 matmuls run warm.
        warm_w = const.tile([128, 128], BF16)
        nc.vector.memset(warm_w[:], 0.0)

        # ================= phase 1: GN + V^T + Q/K of pair 0 ================
        with ExitStack() as ph1:
            xsqp = ph1.enter_context(tc.tile_pool(name="xsqp", bufs=2))
            warm_ps = ph1.enter_context(
                tc.tile_pool(name="warm_ps", bufs=1, space="PSUM"))
            ps_w = warm_ps.tile([128, 128], F32, tag="warm")
            for _ in range(48):
                nc.tensor.matmul(ps_w[:], warm_w[:], warm_w[:],
                                 start=True, stop=True)
            gn_es = ExitStack()
            gn_ps = gn_es.enter_context(
                tc.tile_pool(name="gn_ps", bufs=1, space="PSUM"))
            small_ps = gn_es.enter_context(
                tc.tile_pool(name="small_ps", bufs=2, space="PSUM"))

            # ---------- GroupNorm stats ----------
            # per-channel sum (DVE accumulate) and sum of squares (ACT Square
            # accumulate); a tiny f32r matmul against the group map then does
            # the cross-partition group reduction.
            s12 = gnp.tile([128, 2 * NCC], F32)
            for cc in range(NCC):
                scr = xsqp.tile([128, S], BF16)
                nc.vector.scalar_tensor_tensor(
                    out=scr[:], in0=x_sb[cc][:], scalar=1.0, in1=x_sb[cc][:],
                    op0=mybir.AluOpType.mult, op1=mybir.AluOpType.bypass,
                    accum_out=s12[:, 2 * cc:2 * cc + 1])
                scr2 = xsqp.tile([128, S], BF16)
                nc.scalar.activation(
                    out=scr2[:], in_=x_sb[cc][:], func=AF.Square,
                    accum_out=s12[:, 2 * cc + 1:2 * cc + 2])
            s12r = gnp.tile([128, 2 * NCC], F32R)
            nc.vector.tensor_copy(s12r[:], s12[:])
            ps_g = gn_ps.tile([NG, 2], F32, tag="ps_g")
            for cc in range(NCC):
                nc.tensor.matmul(
                    ps_g[:], gsum_sb[:, NG * cc:NG * (cc + 1)],
                    s12r[:, 2 * cc:2 * cc + 2],
                    start=(cc == 0), stop=(cc == NCC - 1))
            inv_n = 1.0 / (GS * S)
            mean_g = gnp.tile([NG, 1], F32)
            nc.vector.tensor_scalar(out=mean_g[:], in0=ps_g[:, 0:1],
                                    scalar1=inv_n,
                                    scalar2=None, op0=mybir.AluOpType.mult)
            ex2 = gnp.tile([NG, 1], F32)
            nc.vector.tensor_scalar(out=ex2[:], in0=ps_g[:, 1:2],
                                    scalar1=inv_n,
                                    scalar2=None, op0=mybir.AluOpType.mult)
            var_g = gnp.tile([NG, 1], F32)
            # var = E[x^2] - mean^2
            nc.vector.scalar_tensor_tensor(
                out=var_g[:], in0=mean_g[:], scalar=-1.0, in1=mean_g[:],
                op0=mybir.AluOpType.mult, op1=mybir.AluOpType.mult)
            nc.vector.tensor_tensor(out=var_g[:], in0=ex2[:], in1=var_g[:],
                                    op=mybir.AluOpType.add)
            # rstd = exp(-0.5 * ln(var + eps)); ln+exp share one ACT table set
            eps_sb = gnp.tile([NG, 1], F32)
            nc.vector.memset(eps_sb[:], EPS)
            lnv = gnp.tile([NG, 1], F32)
            nc.scalar.activation(out=lnv[:], in_=var_g[:], func=AF.Ln,
                                 bias=eps_sb[:], scale=1.0)
            # stats_r[:, 0] = rstd, stats_r[:, 1] = mean  (N=2 matmul rhs)
            stats_r = gnp.tile([NG, 2], F32R)
            nc.scalar.activation(out=stats_r[:, 0:1], in_=lnv[:], func=AF.Exp,
                                 bias=0.0, scale=-0.5)
            nc.vector.tensor_copy(stats_r[:, 1:2], mean_g[:])

            # per-channel rstd/mean via tiny matmuls against the group map
            rstd_c = gnp.tile([128, NCC], F32)
            mean_c = gnp.tile([128, NCC], F32)
            for cc in range(NCC):
                ps_a = small_ps.tile([128, 2], F32, tag="alpha")
                nc.tensor.matmul(ps_a[:],
                                 gexp_sb[:, 128 * cc:128 * (cc + 1)],
                                 stats_r[:], start=True, stop=True)
                nc.vector.tensor_copy(rstd_c[:, cc:cc + 1], ps_a[:, 0:1])
                nc.vector.tensor_copy(mean_c[:, cc:cc + 1], ps_a[:, 1:2])
            # second HAM-warmup burst: bridges the PE-idle window between the
            # tiny stats matmuls and the first V^T matmuls.
            for _ in range(36):
                nc.tensor.matmul(ps_w[:], warm_w[:], warm_w[:],
                                 start=True, stop=True)
            alpha = gnp.tile([128, NCC], F32)
            nc.vector.tensor_tensor(out=alpha[:], in0=rstd_c[:], in1=w4_sb,
                                    op=mybir.AluOpType.mult)
            beta = gnp.tile([128, NCC], F32)
            nc.vector.tensor_tensor(out=beta[:], in0=alpha[:], in1=mean_c[:],
                                    op=mybir.AluOpType.mult)
            nc.vector.tensor_tensor(out=beta[:], in0=b4_sb, in1=beta[:],
                                    op=mybir.AluOpType.subtract)

            # ---------- GN apply (fp8 out, DoubleRow layout) ----------
            for cc in range(NCC):
                nc.vector.tensor_scalar(
                    out=xn_sb[:, cc // 2, cc % 2, :], in0=x_sb[cc][:],
                    scalar1=alpha[:, cc:cc + 1], scalar2=beta[:, cc:cc + 1],
                    op0=mybir.AluOpType.mult, op1=mybir.AluOpType.add)

            gn_es.close()
            qkv_ps = ph1.enter_context(
                tc.tile_pool(name="qkv_ps", bufs=2, space="PSUM"))

            # ---------- V^T (pre-transposed): out[s, vch], DoubleRow ----------
            for sc in range(NSC):
                ps_v = qkv_ps.tile([128, 512], F32, tag="psv")
                for c2 in range(NC2):
                    nc.tensor.matmul(
                        ps_v[:],
                        xn_sb[:, c2, :, 128 * sc:128 * (sc + 1)],
                        qkvw_sb[:, c2, :, 1024:1536],
                        start=(c2 == 0), stop=(c2 == NC2 - 1),
                        perf_mode=DR)
                vT_v = vT_sb[sc][:].rearrange("p (h u) -> p h u", u=65)
                nc.vector.tensor_tensor(
                    out=vT_v[:, :, 0:64],
                    in0=ps_v[:].rearrange("p (h u) -> p h u", u=64),
                    in1=vb_sb[:].rearrange("p (h u) -> p h u", u=64),
                    op=mybir.AluOpType.add)
                nc.vector.memset(vT_v[:, :, 64:65], 1.0)

            # ---------- Q and K of pair 0, channel-major, DoubleRow ----------
            for oc in range(1):
                ps_q = qkv_ps.tile([128, S], F32, tag="psqk")
                for c2 in range(NC2):
                    for qn in range(NQ):
                        nc.tensor.matmul(
                            ps_q[:, 512 * qn:512 * (qn + 1)],
                            qkvw_sb[:, c2, :, 128 * oc:128 * (oc + 1)],
                            xn_sb[:, c2, :, 512 * qn:512 * (qn + 1)],
                            start=(c2 == 0), stop=(c2 == NC2 - 1),
                            perf_mode=DR)
                nc.scalar.activation(out=q_sb[oc][:], in_=ps_q[:],
                                     func=AF.Identity,
                                     bias=qb_sb[:, oc:oc + 1], scale=1.0)
                ps_k = qkv_ps.tile([128, S], F32, tag="psqk")
                for c2 in range(NC2):
                    for qn in range(NQ):
                        nc.tensor.matmul(
                            ps_k[:, 512 * qn:512 * (qn + 1)],
                            qkvw_sb[:, c2, :, 512 + 128 * oc:512 + 128 * (oc + 1)],
                            xn_sb[:, c2, :, 512 * qn:512 * (qn + 1)],
                            start=(c2 == 0), stop=(c2 == NC2 - 1),
                            perf_mode=DR)
                nc.scalar.activation(out=k_sb[oc][:], in_=ps_k[:],
                                     func=AF.Identity,
                                     bias=qb_sb[:, 4 + oc:5 + oc], scale=1.0)

        # ================= phase 2: attention ================
        # Per head pair p: scores^T / exp / attn@[V|1] pipelined per
        # (qn, kc). Q/K DoubleRow matmuls of pair p+1 ride along inside
        # the kc2 loop (one per step, sharing the scores psum pool slots)
        # so the PE stays dense.
        with ExitStack() as ph_att:
            expp = ph_att.enter_context(tc.tile_pool(name="expp", bufs=3))
            rawp = ph_att.enter_context(tc.tile_pool(name="rawp", bufs=2))
            rbp = ph_att.enter_context(tc.tile_pool(name="rbp", bufs=2))
            recp = ph_att.enter_context(tc.tile_pool(name="recp", bufs=2))
            sc_ps = ph_att.enter_context(
                tc.tile_pool(name="sc_ps", bufs=3, space="PSUM"))
            av_ps = ph_att.enter_context(
                tc.tile_pool(name="av_ps", bufs=1, space="PSUM"))

            for p in range(NH // 2):
                hA, hB = 2 * p, 2 * p + 1
                for qn in range(NQ):
                    # next pair's Q (during qn0) or K (during qn1) rides along
                    nxt = p + 1
                    if nxt < NH // 2:
                        ps_nxt = sc_ps.tile([128, S], F32, tag="sc",
                                            name=f"psnxt{p}_{qn}")
                        woff = 128 * nxt if qn == 0 else 512 + 128 * nxt
                    # exp outputs: ACT chunks -> exact exp, bf16 (fp8 out
                    # would cost +160ns per activation); DVE chunks -> int16
                    # Schraudolph bits (bitcast bf16)
                    exp_t = expp.tile([128, (NSC - NDV) * S], BF16, tag="exp")
                    sch_t = (expp.tile([128, NDV * S], I16, tag="sch",
                                       name="sch_t") if NDV else None)
                    ps_av = av_ps.tile([65, S], F32, tag="av")
                    ps_avA = ps_av[:, 0:512]
                    ps_avB = ps_av[:, 512:1024]
                    for kc in range(NSC):
                        kc2, sub = kc // 2, kc % 2
                        # scores^T chunk for both heads (row-tiled pair)
                        ps_s = sc_ps.tile([128, S], F32, tag="sc")
                        nc.tensor.matmul(
                            ps_s[:, 0:512],
                            k_sb[p][0:64, 128 * kc:128 * (kc + 1)],
                            q_sb[p][0:64, 512 * qn:512 * (qn + 1)],
                            start=True, stop=True, tile_position=(0, 0))
                        nc.tensor.matmul(
                            ps_s[:, 512:1024],
                            k_sb[p][64:128, 128 * kc:128 * (kc + 1)],
                            q_sb[p][64:128, 512 * qn:512 * (qn + 1)],
                            start=True, stop=True, tile_position=(64, 0))
                        # one Q/K DoubleRow matmul of the next pair per 2 kc
                        if nxt < NH // 2 and sub == 0:
                            c2, qq = kc2 // 2, kc2 % 2
                            nc.tensor.matmul(
                                ps_nxt[:, 512 * qq:512 * (qq + 1)],
                                qkvw_sb[:, c2, :, woff:woff + 128],
                                xn_sb[:, c2, :, 512 * qq:512 * (qq + 1)],
                                start=(c2 == 0), stop=(c2 == NC2 - 1),
                                perf_mode=DR)
                        if kc < NSC - NDV:
                            # exp of both heads chunk in one ACT pass
                            # (1/sqrt(hd) scale fused)
                            ka = kc
                            eo = exp_t[:, S * ka:S * (ka + 1)]
                            nc.scalar.activation(out=eo, in_=ps_s[:],
                                                 func=AF.Exp,
                                                 bias=0.0, scale=SCALE)
                        else:
                            # Schraudolph exp on the DVE: int16 bits, read
                            # back as bf16 by the attn@V matmuls
                            kd = kc - (NSC - NDV)
                            nc.vector.tensor_scalar(
                                out=sch_t[:, S * kd:S * (kd + 1)],
                                in0=ps_s[:],
                                scalar1=SCH1, scalar2=SCH2,
                                op0=mybir.AluOpType.mult,
                                op1=mybir.AluOpType.add)
                            eo = sch_t[:, S * kd:S * (kd + 1)].bitcast(BF16)
                        # attn @ [V | 1] per head: rows 0-63 = attn@V,
                        # row 64 = softmax denominator
                        nc.tensor.matmul(
                            ps_avA[:], vT_sb[kc][:, 65 * hA:65 * (hA + 1)],
                            eo[:, 0:512],
                            start=(kc == 0), stop=(kc == NSC - 1))
                        nc.tensor.matmul(
                            ps_avB[:], vT_sb[kc][:, 65 * hB:65 * (hB + 1)],
                            eo[:, 512:1024],
                            start=(kc == 0), stop=(kc == NSC - 1))

                    # next pair's Q/K eviction (+bias)
                    if nxt < NH // 2:
                        dst = q_sb[nxt] if qn == 0 else k_sb[nxt]
                        boff = nxt if qn == 0 else 4 + nxt
                        nc.vector.tensor_scalar(
                            out=dst[:], in0=ps_nxt[:],
                            scalar1=qb_sb[:, boff:boff + 1], scalar2=None,
                            op0=mybir.AluOpType.add)

                    # evict attn@V + denominators raw (frees the psum banks)
                    raw = rawp.tile([65, S], F32, tag="raw")
                    nc.vector.tensor_copy(raw[:], ps_av[:])
                    if p < NH // 2 - 1:
                        # steady state: spread [1,1024] over 128 partitions,
                        # fast DVE reciprocal, DMA roundtrip broadcast;
                        # normalize on GpSimd (latency hidden by later units)
                        d128 = recp.tile([128, 8], F32, tag="d128")
                        nc.sync.dma_start(d128[:], raw[64:65, :])
                        r128 = recp.tile([128, 8], F32, tag="r128")
                        rscr = recp.tile([128, 8], F32, tag="rscr")
                        nc.vector.reciprocal_approx_accurate(
                            out=r128[:], in_=d128[:], scratch=rscr[:])
                        r128v = recip_d[p][qn].rearrange(
                            "h (x f) -> (h x) f", f=8)
                        nc.sync.dma_start(r128v, r128[:])
                        rbA = rbp.tile([64, 512], F32, tag="rbA")
                        rbB = rbp.tile([64, 512], F32, tag="rbB")
                        for rb_t, h in ((rbA, 0), (rbB, 1)):
                            rsrc = recip_d[p][qn][h]  # [512]
                            rsrc_b = bass.AP(tensor=rsrc.tensor,
                                             offset=rsrc.offset,
                                             ap=[[0, 64], list(rsrc.ap[0])])
                            nc.sync.dma_start(rb_t[:], rsrc_b)
                        nc.gpsimd.tensor_tensor(
                            out=an_sb[0:64, p // 2, p % 2,
                                      512 * qn:512 * (qn + 1)],
                            in0=raw[0:64, 0:512], in1=rbA[:],
                            op=mybir.AluOpType.mult)
                        nc.gpsimd.tensor_tensor(
                            out=an_sb[64:128, p // 2, p % 2,
                                      512 * qn:512 * (qn + 1)],
                            in0=raw[0:64, 512:1024], in1=rbB[:],
                            op=mybir.AluOpType.mult)
                    else:
                        # pair 3 gates the proj tail: DMA-free path. recip =
                        # exp(-ln(den)) on the (idle) ACT engine, broadcast
                        # across 64 partitions with a K=1 matmul into the
                        # unused ride-along psum, normalize on the DVE.
                        lnden = recp.tile([1, S], F32, tag="lnden")
                        nc.scalar.activation(out=lnden[:], in_=raw[64:65, :],
                                             func=AF.Ln, bias=0.0, scale=1.0)
                        rrow = recp.tile([1, S], F32R, tag="rrow")
                        nc.scalar.activation(out=rrow[:], in_=lnden[:],
                                             func=AF.Exp, bias=0.0,
                                             scale=-1.0)
                        ps_rb = nxt_ps.tile([128, S], F32, tag="nxt",
                                            name=f"psrb{qn}")
                        rrow_r = rrow[:]
                        for half in range(2):
                            nc.tensor.matmul(
                                ps_rb[0:64, 512 * half:512 * (half + 1)],
                                ones64_r[:],
                                rrow_r[:, 512 * half:512 * (half + 1)],
                                start=True, stop=True)
                        nc.vector.tensor_tensor(
                            out=an_sb[0:64, p // 2, p % 2,
                                      512 * qn:512 * (qn + 1)],
                            in0=raw[0:64, 0:512], in1=ps_rb[0:64, 0:512],
                            op=mybir.AluOpType.mult)
                        nc.vector.tensor_tensor(
                            out=an_sb[64:128, p // 2, p % 2,
                                      512 * qn:512 * (qn + 1)],
                            in0=raw[0:64, 512:1024],
                            in1=ps_rb[0:64, 512:1024],
                            op=mybir.AluOpType.mult)

        # ================= phase 3: proj + bias + residual ================
        with ExitStack() as ph_proj:
            pj_ps = ph_proj.enter_context(
                tc.tile_pool(name="pj_ps", bufs=1, space="PSUM"))
            ps_o = [pj_ps.tile([128, S], F32, tag=f"pso{oc}", name=f"pso{oc}")
                    for oc in range(NCC)]
            for c2 in range(NC2):
                for oc in range(NCC):
                    for qn in range(NQ):
                        nc.tensor.matmul(
                            ps_o[oc][:, 512 * qn:512 * (qn + 1)],
                            pw_sb[:, c2, :, 128 * oc:128 * (oc + 1)],
                            an_sb[:, c2, :, 512 * qn:512 * (qn + 1)],
                            start=(c2 == 0), stop=(c2 == NC2 - 1),
                            perf_mode=DR)
            # evictions: bias + residual fused in one DVE op per chunk,
            # output DMA issued immediately per chunk
            for oc in range(NCC):
                out_t = outp.tile([128, S], F32, tag="out")
                nc.vector.scalar_tensor_tensor(
                    out=out_t[:], in0=ps_o[oc][:],
                    scalar=pb_sb[:, oc:oc + 1], in1=x_sb[oc][:],
                    op0=mybir.AluOpType.add, op1=mybir.AluOpType.add)
                nc.sync.dma_start(y_d[128 * oc:128 * (oc + 1), :], out_t[:])

    nc.finalize()
    return nc


_NC_CACHE = None


def _get_nc():
    global _NC_CACHE
    if _NC_CACHE is None:
        _NC_CACHE = build_nc()
    return _NC_CACHE


def make_in_maps(X, norm_w, norm_b, qkv_w, qkv_b, proj_w, proj_b):
    X = np.asarray(X, dtype=np.float32)
    norm_w = np.asarray(norm_w, dtype=np.float32)
    norm_b = np.asarray(norm_b, dtype=np.float32)
    qkv_w = np.asarray(qkv_w, dtype=np.float32)
    qkv_b = np.asarray(qkv_b, dtype=np.float32)
    proj_w = np.asarray(proj_w, dtype=np.float32)
    proj_b = np.asarray(proj_b, dtype=np.float32)

    # DoubleRow weight layout: [part, c2, sub, out] with
    # c = c2*256 + sub*128 + part
    qkv_w8 = np.ascontiguousarray(
        qkv_w.T.reshape(NC2, 2, 128, 3 * C).transpose(2, 0, 1, 3)
    ).astype(ml_dtypes.float8_e4m3)
    proj_w8 = np.ascontiguousarray(
        proj_w.T.reshape(NC2, 2, 128, C).transpose(2, 0, 1, 3)
    ).astype(ml_dtypes.float8_e4m3)
    gsum = np.zeros((C, NG), np.float32)
    gsum[np.arange(C), np.arange(C) // GS] = 1.0
    gexpT = np.ascontiguousarray(gsum.T)                      # [32, 512]
    w4 = np.ascontiguousarray(norm_w.reshape(NCC, 128).T)     # [128, 4]
    b4 = np.ascontiguousarray(norm_b.reshape(NCC, 128).T)
    qb12 = np.ascontiguousarray(qkv_b.reshape(12, 128).T)     # [128, 12]
    vb_bcast = np.ascontiguousarray(
        np.broadcast_to(qkv_b[2 * C:3 * C], (128, C)))        # [128, 512]
    pb4 = np.ascontiguousarray(proj_b.reshape(NCC, 128).T)

    small_consts = np.ascontiguousarray(
        np.concatenate([w4, b4, pb4, qb12], axis=1))
    shared = {
        "qkv_w8": qkv_w8, "proj_w8": proj_w8, "gsum": gsum, "gexpT": gexpT,
        "small_consts": small_consts, "vb_bcast": vb_bcast,
    }
    in_maps = []
    for b in range(B):
        m = dict(shared)
        m["x"] = np.ascontiguousarray(X[b].reshape(C, S))
        in_maps.append(m)
    return in_maps


def kernel(X, norm_w, norm_b, qkv_w, qkv_b, proj_w, proj_b):
    nc = _get_nc()
    in_maps = make_in_maps(X, norm_w, norm_b, qkv_w, qkv_b, proj_w, proj_b)
    res = run_bass_kernel_spmd(nc, in_maps, core_ids=list(range(B)))
    out = np.stack([res.results[b]["y"].reshape(C, H, W) for b in range(B)])
    return out.astype(np.float32)
